# revision 2
# baseline (speedup 1.0000x reference)
"""Butterfly permuter kernel for Trainium2 (8 NeuronCores, SPMD data-parallel).

The reference applies 10 butterfly rotation stages along the feature axis
(dim=1024) of x [16384, 1024].  Each row is transformed independently, and the
10 stages compose into a single dense 1024x1024 orthogonal matrix R with
y_rows = x_rows @ R.  We compute R on the host in float64 from `angles`, then
run a tiled matmul on each core:

  per core: x_shard [2048, 1024]
  - DMA x in 2 MiB megatiles [128 part, 4096] (4 row-subtiles of 128 tokens)
  - PE-transpose each [128 tok, 128 dim] block (float32r, via identity) to get
    X^T blocks (contraction dim on partitions), evacuate PSUM->SBUF on ScalarE
  - 16 accumulating float32r matmuls per subtile: psum_y[jh] += XT_kb^T @ R_kb
    (float32r streams 1 cycle/row at N=512 - full PE rate, ~fp32 storage)
  - evacuate y PSUM->SBUF on VectorE, DMA out 2 MiB megatiles

Inputs arrive full-size; sharding is across the token axis (2048 rows/core).
"""

import numpy as np

import concourse.bass as bass
import concourse.mybir as mybir
import concourse.tile as tile
from concourse import bacc
from concourse.bass_utils import run_bass_kernel_spmd

N_CORES = 8
DIM = 1024
NUM_STAGES = 10
N_TOKENS = 16384
TOK_PER_CORE = N_TOKENS // N_CORES  # 2048
SUB = 128  # tokens per subtile (partition dim)
SUBTILES_PER_MEGA = 4
MEGA_ROWS = SUB * SUBTILES_PER_MEGA  # 512 tokens per DMA megatile
N_MEGA = TOK_PER_CORE // MEGA_ROWS  # 4
KB = DIM // 128  # 8 contraction blocks

F32 = mybir.dt.float32
F32R = mybir.dt.float32r


def compose_transform(angles: np.ndarray) -> np.ndarray:
    """Compose the 10 butterfly stages into R (float32) with y = x @ R."""
    y = np.eye(DIM, dtype=np.float64)
    a = np.asarray(angles, dtype=np.float64)
    for s in range(NUM_STAGES):
        span = 2 ** (s + 1)
        half = span // 2
        y = y.reshape(-1, DIM // span, span)
        left, right = y[..., :half], y[..., half:]
        th = a[s].reshape(1, DIM // span, half)
        c, sn = np.cos(th), np.sin(th)
        y = np.concatenate([c * left + sn * right, -sn * left + c * right], -1)
        y = y.reshape(-1, DIM)
    # row t of y is transform(e_t), so transform(x) = x @ y
    return np.ascontiguousarray(y, dtype=np.float32)


def build_bass(reps: int = 1):
    """reps>1 repeats the whole pipeline in one NEFF (for marginal timing)."""
    nc = bacc.Bacc(None, target_bir_lowering=False)
    x = nc.dram_tensor("x", [TOK_PER_CORE, DIM], F32, kind="ExternalInput")
    w = nc.dram_tensor("w", [DIM, DIM], F32, kind="ExternalInput")
    ident = nc.dram_tensor("ident", [128, 128], F32, kind="ExternalInput")
    y = nc.dram_tensor("y", [TOK_PER_CORE, DIM], F32, kind="ExternalOutput")

    n_sub = N_MEGA * SUBTILES_PER_MEGA  # 16 subtiles of 128 tokens

    # Variable-size DMA chunking (in units of 128-token subtiles): small
    # chunks at the start for a fast pipeline ramp, small at the end for a
    # short drain; 2-subtile (1 MiB) chunks in steady state.
    in_chunks = [1, 1, 2, 2, 2, 2, 2, 2, 2]
    out_chunks = [2, 2, 2, 2, 2, 2, 2, 1, 1]
    assert sum(in_chunks) == n_sub and sum(out_chunks) == n_sub
    in_start = [sum(in_chunks[:i]) for i in range(len(in_chunks))]
    out_start = [sum(out_chunks[:i]) for i in range(len(out_chunks))]
    sub_to_in_chunk = {}
    for ci, (st, ln) in enumerate(zip(in_start, in_chunks)):
        for s in range(st, st + ln):
            sub_to_in_chunk[s] = ci
    sub_to_out_chunk = {}
    for ci, (st, ln) in enumerate(zip(out_start, out_chunks)):
        for s in range(st, st + ln):
            sub_to_out_chunk[s] = ci

    with tile.TileContext(nc) as tc:
        with (
            tc.tile_pool(name="const", bufs=1) as const_pool,
            tc.tile_pool(name="wstage", bufs=3) as wstage_pool,
            tc.tile_pool(name="xin", bufs=3) as xin_pool,
            tc.tile_pool(name="xt", bufs=5) as xt_pool,
            tc.tile_pool(name="yout", bufs=3) as yout_pool,
            tc.tile_pool(name="pst", bufs=4, space="PSUM") as pst_pool,
            tc.tile_pool(name="psy", bufs=4, space="PSUM") as psy_pool,
        ):
            # identity goes via the SWDGE ring; the SP ring starts with the
            # first x chunk; W streams in behind it.
            ident_sb = const_pool.tile([128, 128], F32, name="ident_sb")
            nc.gpsimd.dma_start(ident_sb[:], ident[:])

            x_tiles = [None] * len(in_chunks)  # chunk idx -> (tile, start_sub)
            y_tiles = [None] * len(out_chunks)

            def load_chunk(ci):
                st, ln = in_start[ci], in_chunks[ci]
                x_tile = xin_pool.tile([128, ln * DIM], F32, name="x_chunk",
                                       tag="x_chunk",
                                       padded_shape=[128, 2 * DIM])
                r0 = st * SUB
                nc.sync.dma_start(
                    x_tile[:, : ln * DIM].rearrange("p (s c) -> p s c", c=DIM),
                    x[r0 : r0 + ln * SUB, :].rearrange("(s p) c -> p s c", p=128),
                )
                x_tiles[ci] = x_tile

            load_chunk(0)
            first_load_done = True

            # W: DMA [jh][kb] blocks of [128,512] (j-half-major so the first
            # 2 MiB unblocks the first matmul group) on the ACT HWDGE ring,
            # then round fp32 -> f32r on DVE (walrus requires f32r matmul
            # inputs to come from a rounding instruction).
            w_sbr = const_pool.tile([128, KB * DIM], F32R, name="w_sbr")

            def w_off(jh, kb):
                return (jh * KB + kb) * 512

            for jh in range(2):
                for kb in range(KB):
                    w_stage = wstage_pool.tile([128, 512], F32, name="w_stage",
                                               tag="w_stage")
                    nc.sync.dma_start(
                        w_stage[:],
                        w[kb * 128 : (kb + 1) * 128, jh * 512 : (jh + 1) * 512],
                    )
                    off = w_off(jh, kb)
                    nc.vector.tensor_copy(w_sbr[:, off : off + 512], w_stage[:])

            xts = [None] * n_sub

            def emit_transpose(s):
                ci = sub_to_in_chunk[s]
                xcol = (s - in_start[ci]) * DIM
                x_tile = x_tiles[ci]
                ps_t0 = pst_pool.tile([128, 512], F32, name="ps_t0", tag="ps_t")
                ps_t1 = pst_pool.tile([128, 512], F32, name="ps_t1", tag="ps_t")
                for kb in range(KB):
                    dst = ps_t0 if kb < 4 else ps_t1
                    j = (kb % 4) * 128
                    nc.tensor.transpose(
                        dst[:, j : j + 128],
                        x_tile[:, xcol + kb * 128 : xcol + (kb + 1) * 128],
                        ident_sb,
                    )
                xt = xt_pool.tile([128, DIM], F32R, name="xt", tag="xt")
                nc.scalar.copy(xt[:, :512], ps_t0[:])
                nc.scalar.copy(xt[:, 512:], ps_t1[:])
                xts[s] = xt

            def emit_matmul(s, jh):
                co = sub_to_out_chunk[s]
                st, ln = out_start[co], out_chunks[co]
                if s == st and jh == 0:
                    y_tiles[co] = yout_pool.tile(
                        [128, ln * DIM], F32, name="y_chunk", tag="y_chunk",
                        padded_shape=[128, 2 * DIM],
                    )
                y_tile = y_tiles[co]
                ycol = (s - st) * DIM + jh * 512
                xt = xts[s]
                ps_y = psy_pool.tile([128, 512], F32, name="ps_y", tag="ps_y")
                for kb in range(KB):
                    off = (jh * KB + kb) * 512
                    nc.tensor.matmul(
                        ps_y[:],
                        xt[:, kb * 128 : (kb + 1) * 128],
                        w_sbr[:, off : off + 512],
                        start=(kb == 0),
                        stop=(kb == KB - 1),
                    )
                nc.vector.tensor_copy(y_tile[:, ycol : ycol + 512], ps_y[:])
                if s == st + ln - 1 and jh == 1:
                    r0 = st * SUB
                    # y stores go out on the ACT HWDGE ring so they don't
                    # queue ahead of later x loads on the SP ring.
                    nc.scalar.dma_start(
                        y[r0 : r0 + ln * SUB, :].rearrange("(s p) c -> p s c", p=128),
                        y_tile[:, : ln * DIM].rearrange("p (s c) -> p s c", c=DIM),
                    )

            # Skewed software pipeline: transposes run one subtile ahead of
            # the matmuls so the PE never waits on the ScalarE PSUM->SBUF
            # evacuation of its own transpose outputs.
            # Transposes run two subtiles ahead of the matmuls (more PE
            # runway while W streams in), and j-halves are staggered one
            # subtile apart: MM(s, jh0) then MM(s-1, jh1), so subtile 0's
            # jh1 group (which needs the second half of W) doesn't stall
            # the in-order PE stream at startup.
            SKEW = 2
            for _rep in range(reps):
                if not first_load_done:
                    load_chunk(0)
                first_load_done = False
                for p in range(min(SKEW, n_sub)):
                    ci = sub_to_in_chunk[p]
                    if p == in_start[ci] and p > 0:
                        load_chunk(ci)
                    emit_transpose(p)
                for s in range(n_sub):
                    nxt = s + SKEW
                    if nxt < n_sub:
                        ci = sub_to_in_chunk[nxt]
                        if nxt == in_start[ci]:
                            load_chunk(ci)
                        emit_transpose(nxt)
                    emit_matmul(s, 0)
                    if s >= 1:
                        emit_matmul(s - 1, 1)
                emit_matmul(n_sub - 1, 1)
    nc.compile()
    return nc


_NC_CACHE = None


def _get_nc():
    global _NC_CACHE
    if _NC_CACHE is None:
        _NC_CACHE = build_bass()
    return _NC_CACHE


def make_in_maps(x: np.ndarray, angles: np.ndarray) -> list[dict]:
    """Host-side sharding: per-core input dict list."""
    x = np.ascontiguousarray(np.asarray(x, dtype=np.float32))
    w = compose_transform(angles)
    ident = np.eye(128, dtype=np.float32)
    in_maps = []
    for c in range(N_CORES):
        in_maps.append(
            {
                "x": x[c * TOK_PER_CORE : (c + 1) * TOK_PER_CORE],
                "w": w,
                "ident": ident,
            }
        )
    return in_maps


def gather_out(per_core_results: list[dict]) -> np.ndarray:
    """Host-side unshard: per-core output dicts -> full fp32 output."""
    return np.concatenate(
        [per_core_results[c]["y"] for c in range(N_CORES)], axis=0
    ).astype(np.float32, copy=False)


def run(x: np.ndarray, angles: np.ndarray, trace: bool = False):
    """Run on 8 cores; returns (y_full, BassKernelResults)."""
    nc = _get_nc()
    in_maps = make_in_maps(x, angles)
    res = run_bass_kernel_spmd(
        nc, in_maps, core_ids=list(range(N_CORES)), trace=trace
    )
    y = gather_out(res.results)
    return y, res


def kernel(x: np.ndarray, angles: np.ndarray) -> np.ndarray:
    y, _ = run(x, angles, trace=False)
    return y



# revision 8
# speedup vs baseline: 1.8293x; 1.8293x over previous
"""Butterfly permuter kernel for Trainium2 (8 NeuronCores, SPMD data-parallel).

The reference applies 10 butterfly rotation stages along the feature axis
(dim=1024) of x [16384, 1024].  Stage s pairs features differing in bit s, so
the 10 stages factor by feature-bit locality:

  * stages 0-6 touch bits b0..b6  -> dense 128x128 blocks, block-diagonal in
    the natural feature tiling (tile T = f >> 7)
  * stages 7-9 touch bits b7..b9  -> block-diagonal in the STRIDED tiling
    (partition q = f >> 3, tile r = f & 7), where each 128x128 per-r matrix
    M2_r[q', q] is nonzero only for q' == q (mod 16)

Device pipeline per core (all math on device; host only reshapes/casts):
  pass 1  stages 0-6 with the DATA as the stationary operand:
            out[t, f'] = sum_f x[f, t] * M1_T[f', f]   (output token-major)
  pass 2  PE transposes of stride-8 feature slices -> q-major layout
  pass 3  stages 7-9 matrix-stationary per r-tile:
            yq_r[q', t] = sum_q M2_r[q', q] * zq_r[q, t]

This is ~48k PE cycles/rep vs ~147k for the dense-1024 matmul formulation.
I/O is fp16 (host casts; fp16 quantization error ~2e-4 rel L2, budget 2e-2):
4 MiB in + 4 MiB out per core per rep = ~23 us at the 360 GB/s DMA roofline.
Host supplies x feature-major ([1024, 2048] per core) and un-permutes the
q-major fp16 output.  PSUM->SBUF evacuations are spread across DVE,
Activation and Pool so no single copy engine becomes the bottleneck.
"""

import numpy as np

import concourse.bass as bass
import concourse.mybir as mybir
import concourse.tile as tile
from concourse import bacc
from concourse.bass_utils import run_bass_kernel_spmd

N_CORES = 8
DIM = 1024
NUM_STAGES = 10
N_TOKENS = 16384
TOK_PER_CORE = N_TOKENS // N_CORES  # 2048
TC = 512  # tokens per chunk (DMA + pass-3 granularity)
NCHUNK = TOK_PER_CORE // TC  # 4
NSUB = TC // 128  # 128-token subchunks per chunk

F16 = mybir.dt.float16
F32 = mybir.dt.float32


def _apply_stages(y, angles, stages):
    y = y.reshape(-1, DIM)
    for stage in stages:
        span = 2 ** (stage + 1)
        half = span // 2
        y = y.reshape(-1, DIM // span, span)
        left, right = y[..., :half], y[..., half:]
        th = angles[stage].reshape(1, DIM // span, half)
        c, s = np.cos(th), np.sin(th)
        y = np.concatenate([c * left + s * right, -s * left + c * right], -1)
        y = y.reshape(-1, DIM)
    return y


def _stage_matrix(angles, stages):
    """M[f_out, f_in] with transform(x) = x @ M.T."""
    return _apply_stages(np.eye(DIM), angles, stages).T


def compose_pass_matrices(angles: np.ndarray):
    """w1 [128, 1024]: w1[p, T*128+j] = M1[T*128+j, T*128+p]  (G1_T = M1_T^T)
    w2 [128, 1024]: w2[q, r*128+q'] = M2[q'*8+r, q*8+r]       (lhsT [K=q, M=q'])
    """
    a = np.asarray(angles, dtype=np.float64)
    M1 = _stage_matrix(a, range(0, 7))
    M2 = _stage_matrix(a, range(7, 10))
    w1 = np.empty((128, DIM), dtype=np.float64)
    for T in range(8):
        blk = M1[T * 128 : (T + 1) * 128, T * 128 : (T + 1) * 128]
        w1[:, T * 128 : (T + 1) * 128] = blk.T  # [f_in, f_out]
    w2 = np.empty((128, DIM), dtype=np.float64)
    for r in range(8):
        fidx = np.arange(r, DIM, 8)
        blk = M2[np.ix_(fidx, fidx)]  # [q', q]
        w2[:, r * 128 : (r + 1) * 128] = blk.T  # [q, q']
    return w1.astype(np.float16), w2.astype(np.float16)


def build_bass(reps: int = 1):
    """reps>1 repeats the whole pipeline in one NEFF (for marginal timing)."""
    nc = bacc.Bacc(None, target_bir_lowering=False)
    xt = nc.dram_tensor("xt", [DIM, TOK_PER_CORE], F16, kind="ExternalInput")
    w1 = nc.dram_tensor("w1", [128, DIM], F16, kind="ExternalInput")
    w2 = nc.dram_tensor("w2", [128, DIM], F16, kind="ExternalInput")
    ident = nc.dram_tensor("ident", [128, 128], F16, kind="ExternalInput")
    yq = nc.dram_tensor("yq", [DIM, TOK_PER_CORE], F16, kind="ExternalOutput")

    # GPSIMD cannot read PSUM, so evacuations split across DVE and ACT.
    # fp16->fp16 copies (pass 2) go to DVE where the packed 2x mode applies;
    # fp32->fp16 copies alternate to balance the two engines' busy time.
    def evac_copy(kind, i, dst, src):
        if kind == "f16":
            nc.vector.tensor_copy(dst, src)
        elif i % 8 < 6:
            nc.scalar.copy(dst, src)
        else:
            nc.vector.tensor_copy(dst, src)

    with tile.TileContext(nc) as tc:
        with (
            tc.tile_pool(name="const", bufs=1) as const_pool,
            tc.tile_pool(name="xin", bufs=3) as xin_pool,
            tc.tile_pool(name="zt", bufs=4) as zt_pool,
            tc.tile_pool(name="zq", bufs=2) as zq_pool,
            tc.tile_pool(name="yout", bufs=2) as yout_pool,
            tc.tile_pool(name="ps1", bufs=3, space="PSUM") as ps1_pool,
            tc.tile_pool(name="ps2", bufs=3, space="PSUM") as ps2_pool,
            tc.tile_pool(name="ps3", bufs=2, space="PSUM") as ps3_pool,
        ):
            ident_sb = const_pool.tile([128, 128], F16, name="ident_sb")
            nc.gpsimd.dma_start(ident_sb[:], ident[:])
            w1_sb = const_pool.tile([128, DIM], F16, name="w1_sb")
            nc.gpsimd.dma_start(w1_sb[:], w1[:])
            w2_sb = const_pool.tile([128, DIM], F16, name="w2_sb")
            nc.gpsimd.dma_start(w2_sb[:], w2[:])

            x_tiles = [None] * NCHUNK
            zq_tiles = [None] * NCHUNK
            ecnt = [0]  # running evac counter for round-robin

            def load_chunk(c):
                x_tile = xin_pool.tile(
                    [128, 8 * TC], F16, name="x_chunk", tag="x_chunk"
                )
                nc.sync.dma_start(
                    x_tile[:].rearrange("p (T t) -> p T t", t=TC),
                    xt[:, c * TC : (c + 1) * TC].rearrange(
                        "(T p) t -> p T t", p=128
                    ),
                )
                x_tiles[c] = x_tile

            def emit_p12(c):
                """Passes 1+2 for one 512-token chunk: fills zq_tiles[c]."""
                x_tile = x_tiles[c]
                zq_tile = zq_pool.tile(
                    [128, 8 * TC], F16, name="zq_chunk", tag="zq_chunk"
                )
                zq_tiles[c] = zq_tile
                zqv = zq_tile[:].rearrange("p (r t) -> p r t", t=TC)
                for sub in range(NSUB):
                    zt_tile = zt_pool.tile([128, DIM], F16, name="zt", tag="zt")
                    for Tg in range(2):
                        ps = ps1_pool.tile([128, 512], F32, name="ps1", tag="ps1")
                        for Ti in range(4):
                            T = Tg * 4 + Ti
                            col = T * TC + sub * 128
                            nc.tensor.matmul(
                                ps[:, Ti * 128 : (Ti + 1) * 128],
                                x_tile[:, col : col + 128],
                                w1_sb[:, T * 128 : (T + 1) * 128],
                                start=True,
                                stop=True,
                            )
                        evac_copy("f32", ecnt[0], zt_tile[:, Tg * 512 : (Tg + 1) * 512], ps[:])
                        ecnt[0] += 1
                    ztv = zt_tile[:].rearrange("p (q r) -> p r q", r=8)
                    for rg in range(2):
                        # fp16 PSUM: transposes must match input dtype, and
                        # the fp16->fp16 evac qualifies for DVE 2x mode.
                        ps = ps2_pool.tile([128, 512], F16, name="ps2", tag="ps2")
                        for ri in range(4):
                            r = rg * 4 + ri
                            nc.tensor.transpose(
                                ps[:, ri * 128 : (ri + 1) * 128],
                                ztv[:, r],
                                ident_sb[:],
                            )
                        evac_copy(
                            "f16",
                            ecnt[0],
                            zqv[
                                :,
                                rg * 4 : (rg + 1) * 4,
                                sub * 128 : (sub + 1) * 128,
                            ],
                            ps[:].rearrange("p (r t) -> p r t", t=128),
                        )
                        ecnt[0] += 1

            def emit_p3(c):
                """Pass 3 + store for one chunk (consumes zq_tiles[c])."""
                zq_tile = zq_tiles[c]
                y_tile = yout_pool.tile(
                    [128, 8 * TC], F16, name="y_chunk", tag="y_chunk"
                )
                for r in range(8):
                    ps = ps3_pool.tile([128, 512], F32, name="ps3", tag="ps3")
                    nc.tensor.matmul(
                        ps[:],
                        w2_sb[:, r * 128 : (r + 1) * 128],
                        zq_tile[:, r * TC : (r + 1) * TC],
                        start=True,
                        stop=True,
                    )
                    evac_copy("f32", ecnt[0], y_tile[:, r * TC : (r + 1) * TC], ps[:])
                    ecnt[0] += 1
                # y stores on the ACT HWDGE ring so they don't queue ahead
                # of later x loads on the SP ring.
                nc.scalar.dma_start(
                    yq[:, c * TC : (c + 1) * TC].rearrange(
                        "(r p) t -> p r t", p=128
                    ),
                    y_tile[:].rearrange("p (r t) -> p r t", t=TC),
                )

            # One-chunk skew: pass 3 of chunk c-1 runs while passes 1+2 of
            # chunk c stream, so the PE never waits on evacuations.
            load_chunk(0)
            for _rep in range(reps):
                for c in range(NCHUNK):
                    nxt = c + 1
                    if nxt < NCHUNK:
                        load_chunk(nxt)
                    elif _rep + 1 < reps:
                        load_chunk(0)
                    emit_p12(c)
                    if c > 0:
                        emit_p3(c - 1)
                emit_p3(NCHUNK - 1)
    nc.compile()
    return nc


_NC_CACHE = None


def _get_nc():
    global _NC_CACHE
    if _NC_CACHE is None:
        _NC_CACHE = build_bass()
    return _NC_CACHE


def make_in_maps(x: np.ndarray, angles: np.ndarray) -> list[dict]:
    """Host-side sharding: token-axis shards, feature-major fp16 layout."""
    x16 = np.asarray(x, dtype=np.float16)
    w1, w2 = compose_pass_matrices(angles)
    ident = np.eye(128, dtype=np.float16)
    in_maps = []
    for c in range(N_CORES):
        shard = x16[c * TOK_PER_CORE : (c + 1) * TOK_PER_CORE]
        in_maps.append(
            {
                "xt": np.ascontiguousarray(shard.T),
                "w1": w1,
                "w2": w2,
                "ident": ident,
            }
        )
    return in_maps


def gather_out(per_core_results: list[dict]) -> np.ndarray:
    """Host-side unshard: un-permute q-major fp16 output to [tok, dim] fp32."""
    shards = []
    for c in range(N_CORES):
        yqc = per_core_results[c]["yq"]  # [1024, 2048], row r*128+q'
        y = (
            yqc.reshape(8, 128, TOK_PER_CORE)
            .transpose(2, 1, 0)
            .reshape(TOK_PER_CORE, DIM)
        )
        shards.append(y)
    return np.concatenate(shards, axis=0).astype(np.float32)


def run(x: np.ndarray, angles: np.ndarray, trace: bool = False):
    """Run on 8 cores; returns (y_full, BassKernelResults)."""
    nc = _get_nc()
    in_maps = make_in_maps(x, angles)
    res = run_bass_kernel_spmd(
        nc, in_maps, core_ids=list(range(N_CORES)), trace=trace
    )
    y = gather_out(res.results)
    return y, res


def kernel(x: np.ndarray, angles: np.ndarray) -> np.ndarray:
    y, _ = run(x, angles, trace=False)
    return y


# revision 26
# speedup vs baseline: 2.0004x; 1.0936x over previous
"""Butterfly permuter kernel for Trainium2 (8 NeuronCores, SPMD data-parallel).

The reference applies 10 butterfly rotation stages along the feature axis
(dim=1024) of x [16384, 1024].  Stage s pairs features differing in bit s, so
the 10 stages factor by feature-bit locality:

  * stages 0-6 touch bits b0..b6  -> dense 128x128 blocks, block-diagonal in
    the natural feature tiling (tile T = f >> 7)
  * stages 7-9 touch bits b7..b9  -> block-diagonal in the STRIDED tiling
    (partition q = f >> 3, tile r = f & 7), where each 128x128 per-r matrix
    M2_r[q', q] is nonzero only for q' == q (mod 16)

Device pipeline per core (all math on device; host only reshapes/casts):
  pass 1  stages 0-6 with the DATA as the stationary operand:
            out[t, f'] = sum_f x[f, t] * M1_T[f', f]   (output token-major)
  pass 2  PE transposes of stride-8 feature slices -> q-major layout
  pass 3  stages 7-9 matrix-stationary per r-tile:
            yq_r[q', t] = sum_q M2_r[q', q] * zq_r[q, t]

This is ~48k PE cycles/rep vs ~147k for the dense-1024 matmul formulation.
I/O is fp16 (host casts; fp16 quantization error ~2e-4 rel L2, budget 2e-2):
4 MiB in + 4 MiB out per core per rep = ~23 us at the 360 GB/s DMA roofline.
Host supplies x feature-major ([1024, 2048] per core) and un-permutes the
q-major fp16 output.  PSUM->SBUF evacuations are spread across DVE,
Activation and Pool so no single copy engine becomes the bottleneck.
"""

import numpy as np

import concourse.bass as bass
import concourse.mybir as mybir
import concourse.tile as tile
from concourse import bacc
from concourse.bass_utils import run_bass_kernel_spmd

N_CORES = 8
DIM = 1024
NUM_STAGES = 10
N_TOKENS = 16384
TOK_PER_CORE = N_TOKENS // N_CORES  # 2048
TC = 512  # tokens per chunk (DMA + pass-3 granularity)
NCHUNK = TOK_PER_CORE // TC  # 4
NSUB = TC // 128  # 128-token subchunks per chunk

F16 = mybir.dt.float16
F32 = mybir.dt.float32

# schedule knobs (tuned via cost-model sim; see sim.py / simgap.py)
EVAC_ACT_OF_16 = 11  # fp32 evac units per 16 that go to ACT (rest DVE)
PS1_BUFS = 2
PS2_BUFS = 2
PS3_BUFS = 1


def _apply_stages(y, angles, stages):
    y = y.reshape(-1, DIM)
    for stage in stages:
        span = 2 ** (stage + 1)
        half = span // 2
        y = y.reshape(-1, DIM // span, span)
        left, right = y[..., :half], y[..., half:]
        th = angles[stage].reshape(1, DIM // span, half)
        c, s = np.cos(th), np.sin(th)
        y = np.concatenate([c * left + s * right, -s * left + c * right], -1)
        y = y.reshape(-1, DIM)
    return y


def _stage_matrix(angles, stages):
    """M[f_out, f_in] with transform(x) = x @ M.T."""
    return _apply_stages(np.eye(DIM), angles, stages).T


def compose_pass_matrices(angles: np.ndarray):
    """w1 [128, 1024]: w1[p, T*128+j] = M1[T*128+j, T*128+p]  (G1_T = M1_T^T)
    w2 [128, 1024]: w2[q, r*128+q'] = M2[q'*8+r, q*8+r]       (lhsT [K=q, M=q'])
    """
    a = np.asarray(angles, dtype=np.float64)
    M1 = _stage_matrix(a, range(0, 7))
    M2 = _stage_matrix(a, range(7, 10))
    w1 = np.empty((128, DIM), dtype=np.float64)
    for T in range(8):
        blk = M1[T * 128 : (T + 1) * 128, T * 128 : (T + 1) * 128]
        w1[:, T * 128 : (T + 1) * 128] = blk.T  # [f_in, f_out]
    w2 = np.empty((128, DIM), dtype=np.float64)
    for r in range(8):
        fidx = np.arange(r, DIM, 8)
        blk = M2[np.ix_(fidx, fidx)]  # [q', q]
        w2[:, r * 128 : (r + 1) * 128] = blk.T  # [q, q']
    return w1.astype(np.float16), w2.astype(np.float16)


def build_bass(reps: int = 1):
    """reps>1 repeats the whole pipeline in one NEFF (for marginal timing)."""
    nc = bacc.Bacc(None, target_bir_lowering=False)
    xt = nc.dram_tensor("xt", [DIM, TOK_PER_CORE], F16, kind="ExternalInput")
    w1 = nc.dram_tensor("w1", [128, DIM], F16, kind="ExternalInput")
    w2 = nc.dram_tensor("w2", [128, DIM], F16, kind="ExternalInput")
    ident = nc.dram_tensor("ident", [128, 128], F16, kind="ExternalInput")
    yq = nc.dram_tensor("yq", [DIM, TOK_PER_CORE], F16, kind="ExternalOutput")

    with tile.TileContext(nc) as tc:
        with (
            tc.tile_pool(name="const", bufs=1) as const_pool,
            tc.tile_pool(name="xin", bufs=4) as xin_pool,
            tc.tile_pool(name="zt", bufs=6) as zt_pool,
            tc.tile_pool(name="zq", bufs=3) as zq_pool,
            tc.tile_pool(name="yout", bufs=3) as yout_pool,
            tc.tile_pool(name="ps1", bufs=3, space="PSUM") as ps1_pool,
            tc.tile_pool(name="ps2", bufs=1, space="PSUM") as ps2_pool,
            tc.tile_pool(name="ps3", bufs=2, space="PSUM") as ps3_pool,
        ):
            ident_sb = const_pool.tile([128, 128], F16, name="ident_sb")
            nc.gpsimd.dma_start(ident_sb[:], ident[:])
            w1_sb = const_pool.tile([128, DIM], F16, name="w1_sb")
            nc.gpsimd.dma_start(w1_sb[:], w1[:])
            w2_sb = const_pool.tile([128, DIM], F16, name="w2_sb")
            nc.gpsimd.dma_start(w2_sb[:], w2[:])

            NSLOT = 16  # 128-token subchunks per rep
            x_tiles = {}
            zt_tiles = {}
            ps2_tiles = {}
            zq_tiles = {}
            y_tiles = {}
            # GPSIMD cannot read PSUM, so evacuations go to ACT and DVE.
            # fp32 units round-robin 11:5 ACT:DVE (22:10 per rep balances
            # both engines at ~22.6 us); fp16 pass-2 units always on DVE
            # where the packed 2x mode applies.
            ecnt = [0]

            def evac_f32(dst, src):
                i = ecnt[0]
                ecnt[0] += 1
                if i % 16 < EVAC_ACT_OF_16:
                    nc.scalar.copy(dst, src)
                else:
                    nc.vector.tensor_copy(dst, src)

            def load_chunk(gc):
                """gc: global chunk index (rep*NCHUNK + c)."""
                c = gc % NCHUNK
                x_tile = xin_pool.tile(
                    [128, 8 * TC], F16, name="x_chunk", tag="x_chunk"
                )
                nc.sync.dma_start(
                    x_tile[:].rearrange("p (T t) -> p T t", t=TC),
                    xt[:, c * TC : (c + 1) * TC].rearrange(
                        "(T p) t -> p T t", p=128
                    ),
                )
                x_tiles[gc] = x_tile

            def emit_mm1(s):
                """Pass 1 for 128-token sub s: 8 data-stationary matmuls."""
                gc, sub = s // NSUB, s % NSUB
                x_tile = x_tiles[gc]
                psa = ps1_pool.tile([128, 512], F32, name="ps1a", tag="ps1")
                psb = ps1_pool.tile([128, 512], F32, name="ps1b", tag="ps1")
                for T in range(8):
                    ps = psa if T < 4 else psb
                    col = T * TC + sub * 128
                    nc.tensor.matmul(
                        ps[:, (T % 4) * 128 : (T % 4 + 1) * 128],
                        x_tile[:, col : col + 128],
                        w1_sb[:, T * 128 : (T + 1) * 128],
                        start=True,
                        stop=True,
                    )
                zt_tile = zt_pool.tile([128, DIM], F16, name="zt", tag="zt")
                if s % 4 == 3:
                    nc.vector.tensor_copy(zt_tile[:, :512], psa[:])
                else:
                    nc.scalar.copy(zt_tile[:, :512], psa[:])
                nc.vector.tensor_copy(zt_tile[:, 512:], psb[:])
                zt_tiles[s] = zt_tile
                if gc + 1 in x_tiles and sub == NSUB - 1:
                    pass  # x_tiles cleanup is implicit via pool recycling

            def emit_tr2(s):
                """Pass 2 for sub s: 8 strided transposes into fp16 PSUM."""
                gc, sub = s // NSUB, s % NSUB
                if sub == 0:
                    zq_tiles[gc] = zq_pool.tile(
                        [128, 8 * TC], F16, name="zq_chunk", tag="zq_chunk"
                    )
                zt_tile = zt_tiles.pop(s)
                ztv = zt_tile[:].rearrange("p (q r) -> p r q", r=8)
                ps = ps2_pool.tile([128, 1024], F16, name="ps2", tag="ps2")
                for r in range(8):
                    nc.tensor.transpose(
                        ps[:, r * 128 : (r + 1) * 128], ztv[:, r], ident_sb[:]
                    )
                zqv = zq_tiles[gc][:].rearrange("p (r t) -> p r t", t=TC)
                # fp16->fp16 on DVE: qualifies for the packed 2x mode
                nc.vector.tensor_copy(
                    zqv[:, :, sub * 128 : (sub + 1) * 128],
                    ps[:].rearrange("p (r t) -> p r t", t=128),
                )

            def emit_p3(u):
                """Pass 3 unit u: 2 r-tiles of chunk u//4."""
                gc, rp = u // 4, u % 4
                c = gc % NCHUNK
                if rp == 0:
                    y_tiles[gc] = yout_pool.tile(
                        [128, 8 * TC], F16, name="y_chunk", tag="y_chunk"
                    )
                zq_tile = zq_tiles[gc]
                y_tile = y_tiles[gc]
                ps = ps3_pool.tile([128, 1024], F32, name="ps3", tag="ps3")
                for half in range(2):
                    r = rp * 2 + half
                    nc.tensor.matmul(
                        ps[:, half * 512 : (half + 1) * 512],
                        w2_sb[:, r * 128 : (r + 1) * 128],
                        zq_tile[:, r * TC : (r + 1) * TC],
                        start=True,
                        stop=True,
                    )
                nc.scalar.copy(y_tile[:, rp * 1024 : (rp + 1) * 1024], ps[:])
                if rp == 3:
                    zq_tiles.pop(gc)
                    # y stores on the SWDGE ring: Pool is otherwise idle, so
                    # the DMA setup never lands on a busy engine sequencer.
                    nc.gpsimd.dma_start(
                        yq[:, c * TC : (c + 1) * TC].rearrange(
                            "(r p) t -> p r t", p=128
                        ),
                        y_tiles.pop(gc)[:].rearrange("p (r t) -> p r t", t=TC),
                    )

            # Flat software pipeline across reps: at slot g the PE runs
            # pass-1 matmuls of sub g, pass-2 transposes of sub g-1, and
            # pass-3 of unit g-5 (one 2-r unit per slot; chunk C's zq is
            # complete after slot 4C+4, its units run at slots 4C+5..4C+8).
            # Every PSUM evacuation gets a full slot (~2 us) to drain before
            # the PE needs its buffer again, so the in-order PE stream never
            # waits on DVE/ACT.
            n_sub = NSLOT * reps
            n_chunk = n_sub // NSUB
            load_chunk(0)
            for g in range(n_sub + 6):
                if g < n_sub:
                    gc = g // NSUB
                    if g % NSUB == 1 and gc + 1 < n_chunk:
                        load_chunk(gc + 1)
                    emit_mm1(g)
                if 1 <= g < n_sub + 1:
                    emit_tr2(g - 1)
                if 5 <= g < n_sub + 5:
                    emit_p3(g - 5)
    nc.compile()
    return nc


_NC_CACHE = None


def _get_nc():
    global _NC_CACHE
    if _NC_CACHE is None:
        _NC_CACHE = build_bass()
    return _NC_CACHE


def make_in_maps(x: np.ndarray, angles: np.ndarray) -> list[dict]:
    """Host-side sharding: token-axis shards, feature-major fp16 layout."""
    x16 = np.asarray(x, dtype=np.float16)
    w1, w2 = compose_pass_matrices(angles)
    ident = np.eye(128, dtype=np.float16)
    in_maps = []
    for c in range(N_CORES):
        shard = x16[c * TOK_PER_CORE : (c + 1) * TOK_PER_CORE]
        in_maps.append(
            {
                "xt": np.ascontiguousarray(shard.T),
                "w1": w1,
                "w2": w2,
                "ident": ident,
            }
        )
    return in_maps


def gather_out(per_core_results: list[dict]) -> np.ndarray:
    """Host-side unshard: un-permute q-major fp16 output to [tok, dim] fp32."""
    shards = []
    for c in range(N_CORES):
        yqc = per_core_results[c]["yq"]  # [1024, 2048], row r*128+q'
        y = (
            yqc.reshape(8, 128, TOK_PER_CORE)
            .transpose(2, 1, 0)
            .reshape(TOK_PER_CORE, DIM)
        )
        shards.append(y)
    return np.concatenate(shards, axis=0).astype(np.float32)


def run(x: np.ndarray, angles: np.ndarray, trace: bool = False):
    """Run on 8 cores; returns (y_full, BassKernelResults)."""
    nc = _get_nc()
    in_maps = make_in_maps(x, angles)
    res = run_bass_kernel_spmd(
        nc, in_maps, core_ids=list(range(N_CORES)), trace=trace
    )
    y = gather_out(res.results)
    return y, res


def kernel(x: np.ndarray, angles: np.ndarray) -> np.ndarray:
    y, _ = run(x, angles, trace=False)
    return y


# revision 27
# speedup vs baseline: 2.2064x; 1.1030x over previous
"""Butterfly permuter kernel for Trainium2 (8 NeuronCores, SPMD data-parallel).

The reference applies 10 butterfly rotation stages along the feature axis
(dim=1024) of x [16384, 1024].  Stage s pairs features differing in bit s, so
the 10 stages factor by feature-bit locality:

  * stages 0-6 touch bits b0..b6  -> dense 128x128 blocks, block-diagonal in
    the natural feature tiling (tile T = f >> 7)
  * stages 7-9 touch bits b7..b9  -> block-diagonal in the STRIDED tiling
    (partition q = f >> 3, tile r = f & 7), where each 128x128 per-r matrix
    M2_r[q', q] is nonzero only for q' == q (mod 16)

Device pipeline per core (all math on device; host only reshapes/casts):
  pass 1  stages 0-6 with the DATA as the stationary operand:
            out[t, f'] = sum_f x[f, t] * M1_T[f', f]   (output token-major)
  pass 2  PE transposes of stride-8 feature slices -> q-major layout
  pass 3  stages 7-9 matrix-stationary per r-tile:
            yq_r[q', t] = sum_q M2_r[q', q] * zq_r[q, t]

This is ~48k PE cycles/rep vs ~147k for the dense-1024 matmul formulation.
I/O is fp16 (host casts; fp16 quantization error ~2e-4 rel L2, budget 2e-2):
4 MiB in + 4 MiB out per core per rep = ~23 us at the 360 GB/s DMA roofline.
Host supplies x feature-major ([1024, 2048] per core) and un-permutes the
q-major fp16 output.  PSUM->SBUF evacuations are spread across DVE,
Activation and Pool so no single copy engine becomes the bottleneck.
"""

import numpy as np

import concourse.bass as bass
import concourse.mybir as mybir
import concourse.tile as tile
from concourse import bacc
from concourse.bass_utils import run_bass_kernel_spmd

N_CORES = 8
DIM = 1024
NUM_STAGES = 10
N_TOKENS = 16384
TOK_PER_CORE = N_TOKENS // N_CORES  # 2048
TC = 512  # tokens per chunk (DMA + pass-3 granularity)
NCHUNK = TOK_PER_CORE // TC  # 4
NSUB = TC // 128  # 128-token subchunks per chunk

F16 = mybir.dt.float16
F32 = mybir.dt.float32

# schedule knobs (tuned via cost-model sim; see sim.py / simgap.py)
EVAC_ACT_OF_16 = 11  # fp32 evac units per 16 that go to ACT (rest DVE)
PS1_BUFS = 2
PS2_BUFS = 2
PS3_BUFS = 1


def _apply_stages(y, angles, stages):
    y = y.reshape(-1, DIM)
    for stage in stages:
        span = 2 ** (stage + 1)
        half = span // 2
        y = y.reshape(-1, DIM // span, span)
        left, right = y[..., :half], y[..., half:]
        th = angles[stage].reshape(1, DIM // span, half)
        c, s = np.cos(th), np.sin(th)
        y = np.concatenate([c * left + s * right, -s * left + c * right], -1)
        y = y.reshape(-1, DIM)
    return y


def _stage_matrix(angles, stages):
    """M[f_out, f_in] with transform(x) = x @ M.T."""
    return _apply_stages(np.eye(DIM), angles, stages).T


def compose_pass_matrices(angles: np.ndarray):
    """w1 [128, 1024]: w1[p, T*128+j] = M1[T*128+j, T*128+p]  (G1_T = M1_T^T)
    w2 [128, 1024]: w2[q, r*128+q'] = M2[q'*8+r, q*8+r]       (lhsT [K=q, M=q'])
    """
    a = np.asarray(angles, dtype=np.float64)
    M1 = _stage_matrix(a, range(0, 7))
    M2 = _stage_matrix(a, range(7, 10))
    w1 = np.empty((128, DIM), dtype=np.float64)
    for T in range(8):
        blk = M1[T * 128 : (T + 1) * 128, T * 128 : (T + 1) * 128]
        w1[:, T * 128 : (T + 1) * 128] = blk.T  # [f_in, f_out]
    w2 = np.empty((128, DIM), dtype=np.float64)
    for r in range(8):
        fidx = np.arange(r, DIM, 8)
        blk = M2[np.ix_(fidx, fidx)]  # [q', q]
        w2[:, r * 128 : (r + 1) * 128] = blk.T  # [q, q']
    return w1.astype(np.float16), w2.astype(np.float16)


def build_bass(reps: int = 1):
    """reps>1 repeats the whole pipeline in one NEFF (for marginal timing)."""
    nc = bacc.Bacc(None, target_bir_lowering=False)
    xt = nc.dram_tensor("xt", [DIM, TOK_PER_CORE], F16, kind="ExternalInput")
    w1 = nc.dram_tensor("w1", [128, DIM], F16, kind="ExternalInput")
    w2 = nc.dram_tensor("w2", [128, DIM], F16, kind="ExternalInput")
    ident = nc.dram_tensor("ident", [128, 128], F16, kind="ExternalInput")
    yq = nc.dram_tensor("yq", [DIM, TOK_PER_CORE], F16, kind="ExternalOutput")

    with tile.TileContext(nc) as tc:
        with (
            tc.tile_pool(name="const", bufs=1) as const_pool,
            tc.tile_pool(name="xin", bufs=4) as xin_pool,
            tc.tile_pool(name="zt", bufs=6) as zt_pool,
            tc.tile_pool(name="zq", bufs=3) as zq_pool,
            tc.tile_pool(name="yout", bufs=3) as yout_pool,
            tc.tile_pool(name="ps1", bufs=3, space="PSUM") as ps1_pool,
            tc.tile_pool(name="ps2", bufs=1, space="PSUM") as ps2_pool,
            tc.tile_pool(name="ps3", bufs=2, space="PSUM") as ps3_pool,
        ):
            ident_sb = const_pool.tile([128, 128], F16, name="ident_sb")
            nc.gpsimd.dma_start(ident_sb[:], ident[:])
            w1_sb = const_pool.tile([128, DIM], F16, name="w1_sb")
            nc.gpsimd.dma_start(w1_sb[:], w1[:])
            w2_sb = const_pool.tile([128, DIM], F16, name="w2_sb")
            nc.gpsimd.dma_start(w2_sb[:], w2[:])

            NSLOT = 16  # 128-token subchunks per rep
            x_tiles = {}
            zt_tiles = {}
            ps2_tiles = {}
            zq_tiles = {}
            y_tiles = {}
            # GPSIMD cannot read PSUM, so evacuations go to ACT and DVE.
            # fp32 units round-robin 11:5 ACT:DVE (22:10 per rep balances
            # both engines at ~22.6 us); fp16 pass-2 units always on DVE
            # where the packed 2x mode applies.
            ecnt = [0]

            def evac_f32(dst, src):
                i = ecnt[0]
                ecnt[0] += 1
                if i % 16 < EVAC_ACT_OF_16:
                    nc.scalar.copy(dst, src)
                else:
                    nc.vector.tensor_copy(dst, src)

            def load_chunk(gc):
                """gc: global chunk index (rep*NCHUNK + c)."""
                c = gc % NCHUNK
                x_tile = xin_pool.tile(
                    [128, 8 * TC], F16, name="x_chunk", tag="x_chunk"
                )
                nc.sync.dma_start(
                    x_tile[:].rearrange("p (T t) -> p T t", t=TC),
                    xt[:, c * TC : (c + 1) * TC].rearrange(
                        "(T p) t -> p T t", p=128
                    ),
                )
                x_tiles[gc] = x_tile

            def emit_mm1(s):
                """Pass 1 for 128-token sub s: 8 data-stationary matmuls."""
                gc, sub = s // NSUB, s % NSUB
                x_tile = x_tiles[gc]
                psa = ps1_pool.tile([128, 512], F32, name="ps1a", tag="ps1")
                psb = ps1_pool.tile([128, 512], F32, name="ps1b", tag="ps1")
                for T in range(8):
                    ps = psa if T < 4 else psb
                    col = T * TC + sub * 128
                    nc.tensor.matmul(
                        ps[:, (T % 4) * 128 : (T % 4 + 1) * 128],
                        x_tile[:, col : col + 128],
                        w1_sb[:, T * 128 : (T + 1) * 128],
                        start=True,
                        stop=True,
                    )
                zt_tile = zt_pool.tile([128, DIM], F16, name="zt", tag="zt")
                if s % 4 == 3:
                    nc.vector.tensor_copy(zt_tile[:, :512], psa[:])
                else:
                    nc.scalar.copy(zt_tile[:, :512], psa[:])
                nc.vector.tensor_copy(zt_tile[:, 512:], psb[:])
                zt_tiles[s] = zt_tile
                if gc + 1 in x_tiles and sub == NSUB - 1:
                    pass  # x_tiles cleanup is implicit via pool recycling

            def emit_tr2(s):
                """Pass 2 for sub s: 8 strided transposes into fp16 PSUM."""
                gc, sub = s // NSUB, s % NSUB
                if sub == 0:
                    zq_tiles[gc] = zq_pool.tile(
                        [128, 8 * TC], F16, name="zq_chunk", tag="zq_chunk"
                    )
                zt_tile = zt_tiles.pop(s)
                ztv = zt_tile[:].rearrange("p (q r) -> p r q", r=8)
                ps = ps2_pool.tile([128, 1024], F16, name="ps2", tag="ps2")
                for r in range(8):
                    nc.tensor.transpose(
                        ps[:, r * 128 : (r + 1) * 128], ztv[:, r], ident_sb[:]
                    )
                zqv = zq_tiles[gc][:].rearrange("p (r t) -> p r t", t=TC)
                dst = zqv[:, :, sub * 128 : (sub + 1) * 128]
                srcv = ps[:].rearrange("p (r t) -> p r t", t=128)
                if s % 2 == 0:
                    nc.scalar.copy(dst, srcv)
                else:
                    nc.vector.tensor_copy(dst, srcv)

            def emit_p3(u):
                """Pass 3 unit u: 2 r-tiles of chunk u//4."""
                gc, rp = u // 4, u % 4
                c = gc % NCHUNK
                if rp == 0:
                    y_tiles[gc] = yout_pool.tile(
                        [128, 8 * TC], F16, name="y_chunk", tag="y_chunk"
                    )
                zq_tile = zq_tiles[gc]
                y_tile = y_tiles[gc]
                ps = ps3_pool.tile([128, 1024], F32, name="ps3", tag="ps3")
                for half in range(2):
                    r = rp * 2 + half
                    nc.tensor.matmul(
                        ps[:, half * 512 : (half + 1) * 512],
                        w2_sb[:, r * 128 : (r + 1) * 128],
                        zq_tile[:, r * TC : (r + 1) * TC],
                        start=True,
                        stop=True,
                    )
                if (u * 7) % 16 < 9:
                    nc.scalar.copy(y_tile[:, rp * 1024 : (rp + 1) * 1024], ps[:])
                else:
                    nc.vector.tensor_copy(
                        y_tile[:, rp * 1024 : (rp + 1) * 1024], ps[:]
                    )
                if rp == 3:
                    zq_tiles.pop(gc)
                    # y stores on the SWDGE ring: Pool is otherwise idle, so
                    # the DMA setup never lands on a busy engine sequencer.
                    nc.gpsimd.dma_start(
                        yq[:, c * TC : (c + 1) * TC].rearrange(
                            "(r p) t -> p r t", p=128
                        ),
                        y_tiles.pop(gc)[:].rearrange("p (r t) -> p r t", t=TC),
                    )

            # Flat software pipeline across reps: at slot g the PE runs
            # pass-1 matmuls of sub g, pass-2 transposes of sub g-1, and
            # pass-3 of unit g-5 (one 2-r unit per slot; chunk C's zq is
            # complete after slot 4C+4, its units run at slots 4C+5..4C+8).
            # Every PSUM evacuation gets a full slot (~2 us) to drain before
            # the PE needs its buffer again, so the in-order PE stream never
            # waits on DVE/ACT.
            n_sub = NSLOT * reps
            n_chunk = n_sub // NSUB
            load_chunk(0)
            for g in range(n_sub + 6):
                if g < n_sub:
                    gc = g // NSUB
                    if g % NSUB == 1 and gc + 1 < n_chunk:
                        load_chunk(gc + 1)
                    emit_mm1(g)
                if 1 <= g < n_sub + 1:
                    emit_tr2(g - 1)
                if 5 <= g < n_sub + 5:
                    emit_p3(g - 5)
    nc.compile()
    return nc


_NC_CACHE = None


def _get_nc():
    global _NC_CACHE
    if _NC_CACHE is None:
        _NC_CACHE = build_bass()
    return _NC_CACHE


def make_in_maps(x: np.ndarray, angles: np.ndarray) -> list[dict]:
    """Host-side sharding: token-axis shards, feature-major fp16 layout."""
    x16 = np.asarray(x, dtype=np.float16)
    w1, w2 = compose_pass_matrices(angles)
    ident = np.eye(128, dtype=np.float16)
    in_maps = []
    for c in range(N_CORES):
        shard = x16[c * TOK_PER_CORE : (c + 1) * TOK_PER_CORE]
        in_maps.append(
            {
                "xt": np.ascontiguousarray(shard.T),
                "w1": w1,
                "w2": w2,
                "ident": ident,
            }
        )
    return in_maps


def gather_out(per_core_results: list[dict]) -> np.ndarray:
    """Host-side unshard: un-permute q-major fp16 output to [tok, dim] fp32."""
    shards = []
    for c in range(N_CORES):
        yqc = per_core_results[c]["yq"]  # [1024, 2048], row r*128+q'
        y = (
            yqc.reshape(8, 128, TOK_PER_CORE)
            .transpose(2, 1, 0)
            .reshape(TOK_PER_CORE, DIM)
        )
        shards.append(y)
    return np.concatenate(shards, axis=0).astype(np.float32)


def run(x: np.ndarray, angles: np.ndarray, trace: bool = False):
    """Run on 8 cores; returns (y_full, BassKernelResults)."""
    nc = _get_nc()
    in_maps = make_in_maps(x, angles)
    res = run_bass_kernel_spmd(
        nc, in_maps, core_ids=list(range(N_CORES)), trace=trace
    )
    y = gather_out(res.results)
    return y, res


def kernel(x: np.ndarray, angles: np.ndarray) -> np.ndarray:
    y, _ = run(x, angles, trace=False)
    return y
